# revision 81
# baseline (speedup 1.0000x reference)
"""MixLoRA layer kernel for 8 trn2 NeuronCores.

Data-parallel over batch B=8: core c computes sample c's output end to
end. Router A and the CFS content score are computed cooperatively:
core c holds cfs_W[2c:2c+2] and computes partial g_B scores for ALL
samples over its 2 ranks; an AllGather (cheaper than AllReduce) shares
the partials and each core sums its own sample's rows on the PE.

Value path is fp16 (x is pre-transposed+cast on host, output written
fp16 and upcast on host); all routing math stays fp32 so the top-k
selections match the fp32 reference exactly.

The routing work is interleaved into the main x-streaming loop in
program order so the in-order engine queues never stall the DMA
pipeline: the collective and router-B tail land in slots where their
inputs are long since ready.

Hardcoded problem shape:
  B=8, S=2048, IN=4096, OUT=4096, R=16, E=64, fp32 in/out.
"""

import numpy as np

import concourse.mybir as mybir
from concourse import bacc, bass
from concourse.bass_utils import run_bass_kernel_spmd
from concourse.masks import make_identity
from concourse.tile import TileContext

F32 = mybir.dt.float32
F16 = mybir.dt.float16
U32 = mybir.dt.uint32
I32 = mybir.dt.int32

B, S, IN, OUT, R, E = 8, 2048, 4096, 4096, 16, 64
P = 128
NEG = -1.0e30
RPC = R // B  # cfs ranks per core
NIT = IN // P  # 32 i-tiles of 128
NST = S // 512  # 4 s-chunks of 512
NOC = OUT // 512  # 8 o-chunks of 512


def build_nc() -> bass.Bass:
    nc = bacc.Bacc("TRN2", target_bir_lowering=False, debug=False, num_devices=B)

    # ---- DRAM tensors (host-prepped layouts) ----
    xt_d = nc.dram_tensor("xt", [IN, S], F16, kind="ExternalInput")
    qt_d = nc.dram_tensor("qt", [P, NIT * B], F32, kind="ExternalInput")
    wrat_d = nc.dram_tensor("wrat", [P, NIT * E], F32, kind="ExternalInput")
    wrbt_d = nc.dram_tensor("wrbt", [P, NIT * E], F16, kind="ExternalInput")
    qt16_d = nc.dram_tensor("qt16", [P, NIT * B], F16, kind="ExternalInput")
    cfs_d = nc.dram_tensor("cfs", [RPC, P, NIT * E], F16, kind="ExternalInput")
    a_pool_d = nc.dram_tensor("a_pool", [E * R, IN], F16, kind="ExternalInput")
    # B_pool transposed to (R, E, OUT) so lora_B rows gather contiguously
    btp_d = nc.dram_tensor("btp", [R * E, OUT], F16, kind="ExternalInput")
    b_ra_d = nc.dram_tensor("b_ra", [E, 1], F32, kind="ExternalInput")
    b_rb_d = nc.dram_tensor("b_rb", [E, 1], F32, kind="ExternalInput")
    onehot_d = nc.dram_tensor("onehot", [B, 1], F32, kind="ExternalInput")
    sel64_d = nc.dram_tensor("sel64", [B * B, 1], F32, kind="ExternalInput")
    rsel16_d = nc.dram_tensor("rsel16", [R, RPC], F32, kind="ExternalInput")
    rankrow_d = nc.dram_tensor("rankrow", [1, RPC], F32, kind="ExternalInput")
    out_d = nc.dram_tensor("out", [S, OUT], F16, kind="ExternalOutput")
    ar_in_d = nc.dram_tensor("ar_in", [B, E], F32)
    ar_out_d = nc.dram_tensor("ar_out", [B * B, E], F32)

    with TileContext(nc) as tc:
        with (
            tc.tile_pool(name="consts", bufs=1) as consts,
            tc.tile_pool(name="w_pool", bufs=1) as w_pool,
            tc.tile_pool(name="route_sb", bufs=1) as route_sb,
            tc.tile_pool(name="cfs_pool", bufs=2) as cfs_pool,
            tc.tile_pool(name="gpool", bufs=1) as gpool,
            tc.tile_pool(name="x_pool", bufs=8) as x_pool,
            tc.tile_pool(name="delta_pool", bufs=3) as delta_pool,
            tc.tile_pool(name="route_ps", bufs=1, space="PSUM") as route_ps,
            tc.tile_pool(name="cfs_ps_pool", bufs=1, space="PSUM") as cfs_ps_pool,
            tc.tile_pool(name="main_ps_pool", bufs=6, space="PSUM") as main_ps_pool,
        ):
            ident = consts.tile([P, P], F32)
            make_identity(nc, ident)
            ident_h = consts.tile([P, P], F16)
            nc.vector.tensor_copy(ident_h, ident)

            # ---------------- loads: router-A critical path first ----------
            qt_sb = w_pool.tile([P, NIT * B], F32)
            nc.sync.dma_start(out=qt_sb, in_=qt_d[:, :])
            wrat_sb = w_pool.tile([P, NIT * E], F32)
            nc.sync.dma_start(out=wrat_sb, in_=wrat_d[:, :])
            b_ra_sb = route_sb.tile([E, 1], F32)
            nc.scalar.dma_start(out=b_ra_sb, in_=b_ra_d[:, :])
            b_rb_sb = route_sb.tile([E, 1], F32)
            nc.scalar.dma_start(out=b_rb_sb, in_=b_rb_d[:, :])
            onehot_sb = route_sb.tile([B, 1], F32)
            nc.scalar.dma_start(out=onehot_sb, in_=onehot_d[:, :])
            sel64_sb = route_sb.tile([B * B, 1], F32)
            nc.scalar.dma_start(out=sel64_sb, in_=sel64_d[:, :])
            rsel16_sb = route_sb.tile([R, RPC], F32)
            nc.scalar.dma_start(out=rsel16_sb, in_=rsel16_d[:, :])
            rankrow_sb = route_sb.tile([1, RPC], F32)
            nc.scalar.dma_start(out=rankrow_sb, in_=rankrow_d[:, :])

            iota16 = consts.tile([1, 16], I32)
            nc.gpsimd.iota(iota16, pattern=[[1, 16]], base=0, channel_multiplier=0)
            iota16_f = consts.tile([1, 16], F32)
            nc.vector.tensor_copy(iota16_f, iota16)
            iota64 = consts.tile([1, 16], I32)
            nc.gpsimd.iota(iota64, pattern=[[E, 16]], base=0, channel_multiplier=0)
            iota64_f = consts.tile([1, 16], F32)
            nc.vector.tensor_copy(iota64_f, iota64)
            ones_1b = consts.tile([1, B], F32)
            nc.vector.memset(ones_1b, 1.0)
            onehot16_sb = consts.tile([B, 1], F32)
            nc.vector.tensor_scalar_mul(onehot16_sb, onehot_sb, 16.0)

            # PE warm-up: dummy transposes ramp the pstate while DMAs load
            warm_ps = route_ps.tile([P, P], F32, tag="rps", name="warm_ps")
            for _ in range(12):
                nc.tensor.transpose(out=warm_ps, in_=ident, identity=ident)
            # preload the ACT Copy table so M2's first activation copy
            # doesn't pay the 1.3us table load
            act_warm = consts.tile([1, 16], F32)
            nc.scalar.activation(
                act_warm, iota16_f, mybir.ActivationFunctionType.Copy
            )

            # ---------------- router A (all samples) ----------------
            # scores computed [E, B] (moving N=8) to minimize PE time
            gat_ps = route_ps.tile([E, B], F32, tag="rps", name="gat_ps")
            for t in range(NIT):
                nc.tensor.matmul(
                    out=gat_ps,
                    lhsT=wrat_sb[:, E * t : E * (t + 1)],
                    rhs=qt_sb[:, B * t : B * (t + 1)],
                    start=(t == 0),
                    stop=(t == NIT - 1),
                )
            gat_sb = route_sb.tile([E, B], F32)
            nc.vector.tensor_scalar(
                gat_sb, gat_ps, b_ra_sb, scalar2=None, op0=mybir.AluOpType.add
            )
            ga_ps = route_ps.tile([B, E], F32, tag="rps", name="ga_ps")
            nc.tensor.transpose(out=ga_ps, in_=gat_sb, identity=ident[:E, :E])
            ga_sb = route_sb.tile([B, E], F32)
            nc.vector.tensor_copy(ga_sb, ga_ps)

            def topk16(scores_sb, vals_sb, idx_sb, scratch_sb):
                """scores_sb [n,E] fp32 -> idx_sb [n,16] u32 (desc order)."""
                nc.vector.max(out=vals_sb[:, 0:8], in_=scores_sb)
                nc.vector.max_index(
                    out=idx_sb[:, 0:8], in_max=vals_sb[:, 0:8], in_values=scores_sb
                )
                nc.vector.match_replace(
                    out=scratch_sb,
                    in_to_replace=vals_sb[:, 0:8],
                    in_values=scores_sb,
                    imm_value=NEG,
                )
                nc.vector.max(out=vals_sb[:, 8:16], in_=scratch_sb)
                nc.vector.max_index(
                    out=idx_sb[:, 8:16], in_max=vals_sb[:, 8:16], in_values=scratch_sb
                )

            vals_a = route_sb.tile([B, 16], F32)
            idxa = route_sb.tile([B, 16], U32)
            tka_scr = route_sb.tile([B, E], F32)
            topk16(ga_sb, vals_a, idxa, tka_scr)
            idxa_f = route_sb.tile([B, 16], F32)
            nc.vector.tensor_copy(idxa_f, idxa)

            # own-sample offsets [16, 1]: 16*idxA[c, k] + k  (feeds mm1 lhsT,
            # so this chain goes first; onehot16 pre-scales by 16)
            ownrow_ps = route_ps.tile([1, 16], F32, tag="rps", name="ownrow_ps")
            nc.tensor.matmul(
                out=ownrow_ps, lhsT=onehot16_sb, rhs=idxa_f, start=True, stop=True
            )
            own_rows = route_sb.tile([1, 16], F32)
            nc.vector.tensor_add(out=own_rows, in0=ownrow_ps, in1=iota16_f)
            ownoff_ps = route_ps.tile([16, 1], F32, tag="rps", name="ownoff_ps")
            nc.tensor.transpose(out=ownoff_ps, in_=own_rows, identity=ident[:1, :1])
            ownoff = route_sb.tile([16, 1], I32)
            nc.vector.tensor_copy(ownoff, ownoff_ps)
            lora_own = gpool.tile([16, IN], F16, tag="gown")
            nc.gpsimd.indirect_dma_start(
                out=lora_own[:, :],
                out_offset=None,
                in_=a_pool_d[:, :],
                in_offset=bass.IndirectOffsetOnAxis(ap=ownoff[:, 0:1], axis=0),
            )

            # slice offsets [B, RPC]: 16*idxA[b, 2c+j] + (2c+j)
            idxat_ps = route_ps.tile([R, B], F32, tag="rps", name="idxat_ps")
            nc.tensor.transpose(out=idxat_ps, in_=idxa_f, identity=ident[:B, :B])
            idxa_t = route_sb.tile([R, B], F32)
            nc.vector.tensor_copy(idxa_t, idxat_ps)
            soff_ps = route_ps.tile([B, RPC], F32, tag="rps", name="soff_ps")
            nc.tensor.matmul(
                out=soff_ps, lhsT=idxa_t, rhs=rsel16_sb, start=True, stop=False
            )
            nc.tensor.matmul(
                out=soff_ps, lhsT=ones_1b, rhs=rankrow_sb, start=False, stop=True
            )
            soff = route_sb.tile([B, RPC], I32)
            nc.vector.tensor_copy(soff, soff_ps)
            lora_sl = []
            for j in range(RPC):
                t_ = gpool.tile([B, IN], F16, tag=f"gsl{j}")
                nc.gpsimd.indirect_dma_start(
                    out=t_[:, :],
                    out_offset=None,
                    in_=a_pool_d[:, :],
                    in_offset=bass.IndirectOffsetOnAxis(ap=soff[:, j : j + 1], axis=0),
                )
                lora_sl.append(t_)

            # keep the PE visit-frontier busy while the gather transfer
            # queues on the DMA device, so mm1 doesn't start in the slow
            # mid pstate
            # transposes: own rows -> lat16 [128, 16] per i-tile (fp16 mm1 lhsT)
            lat16 = consts.tile([P, R * NIT], F16)
            for g in range(4):
                lat_ps = route_ps.tile([P, 128], F16, tag="rps", name="lat_ps")
                for t8 in range(8):
                    t = g * 8 + t8
                    nc.tensor.transpose(
                        out=lat_ps[:, 16 * t8 : 16 * (t8 + 1)],
                        in_=lora_own[:, P * t : P * (t + 1)],
                        identity=ident_h[:R, :R],
                    )
                nc.vector.tensor_copy(lat16[:, 128 * g : 128 * (g + 1)], lat_ps)

            # ---- deferred routing blocks, interleaved into the M1 loop ----
            cfs_sb = []

            def load_cfs(j):
                t_ = cfs_pool.tile([P, NIT * E], F16, tag="cfs")
                nc.sync.dma_start(out=t_, in_=cfs_d[j])
                cfs_sb.append(t_)

            lat_s = consts.tile([P, R * NIT], F16)

            def cfs_and_collective():
                # slice rows -> lat_s [128, 16] per i-tile (fp32, cols j*8+b)
                for g in range(4):
                    lats_ps = route_ps.tile([P, 128], F16, tag="rps", name="lats_ps")
                    for t8 in range(8):
                        t = g * 8 + t8
                        for j in range(RPC):
                            nc.tensor.transpose(
                                out=lats_ps[
                                    :, 16 * t8 + 8 * j : 16 * t8 + 8 * (j + 1)
                                ],
                                in_=lora_sl[j][:, P * t : P * (t + 1)],
                                identity=ident_h[:B, :B],
                            )
                    nc.vector.tensor_copy(lat_s[:, 128 * g : 128 * (g + 1)], lats_ps)
                # cfs partial scores [E, B] for all samples over own 2 ranks
                cfst_ps = cfs_ps_pool.tile([E, B], F32, tag="cfsps")
                for j in range(RPC):
                    for t in range(NIT):
                        nc.tensor.matmul(
                            out=cfst_ps,
                            lhsT=cfs_sb[j][:, E * t : E * (t + 1)],
                            rhs=lat_s[:, 16 * t + B * j : 16 * t + B * j + B],
                            start=(j == 0 and t == 0),
                            stop=(j == RPC - 1 and t == NIT - 1),
                        )
                cfst_sb = route_sb.tile([E, B], F32)
                nc.vector.tensor_copy(cfst_sb, cfst_ps)
                cfsp_ps = route_ps.tile([B, E], F32, tag="rps", name="cfsp_ps")
                nc.tensor.transpose(out=cfsp_ps, in_=cfst_sb, identity=ident[:E, :E])
                cfs_part = route_sb.tile([B, E], F32)
                nc.vector.tensor_copy(cfs_part, cfsp_ps)
                nc.scalar.dma_start(out=ar_in_d[:, :], in_=cfs_part)
                nc.gpsimd.collective_compute(
                    "AllGather",
                    mybir.AluOpType.bypass,
                    replica_groups=[list(range(B))],
                    ins=[ar_in_d.ap().opt()],
                    outs=[ar_out_d.ap().opt()],
                )

            wrbt_sb = w_pool.tile([P, NIT * E], F16)
            qt16_sb = w_pool.tile([P, NIT * B], F16)

            def load_wrbt():
                nc.sync.dma_start(out=wrbt_sb, in_=wrbt_d[:, :])
                nc.sync.dma_start(out=qt16_sb, in_=qt16_d[:, :])

            gb_sb = route_sb.tile([B, E], F32)

            def gb_linear():
                gbt_ps = cfs_ps_pool.tile([E, B], F32, tag="cfsps", name="gbt_ps")
                for t in range(NIT):
                    nc.tensor.matmul(
                        out=gbt_ps,
                        lhsT=wrbt_sb[:, E * t : E * (t + 1)],
                        rhs=qt16_sb[:, B * t : B * (t + 1)],
                        start=(t == 0),
                        stop=(t == NIT - 1),
                    )
                gbt_sb = route_sb.tile([E, B], F32)
                nc.vector.tensor_scalar(
                    gbt_sb, gbt_ps, b_rb_sb, scalar2=None, op0=mybir.AluOpType.add
                )
                gb_ps = route_ps.tile([B, E], F32, tag="rps", name="gb_ps")
                nc.tensor.transpose(out=gb_ps, in_=gbt_sb, identity=ident[:E, :E])
                nc.vector.tensor_copy(gb_sb, gb_ps)

            lora_b = w_pool.tile([R, OUT], F16)

            def router_b_tail():
                ar_sb = route_sb.tile([B * B, E], F32)
                nc.scalar.dma_start(out=ar_sb, in_=ar_out_d[:, :])
                owncfs_ps = route_ps.tile([1, E], F32, tag="rps", name="owncfs_ps")
                nc.tensor.matmul(
                    out=owncfs_ps, lhsT=sel64_sb, rhs=ar_sb, start=True, stop=True
                )
                ownlin_ps = cfs_ps_pool.tile([1, E], F32, tag="cfsps", name="ownlin")
                nc.tensor.matmul(
                    out=ownlin_ps, lhsT=onehot_sb, rhs=gb_sb, start=True, stop=True
                )
                gb_own = route_sb.tile([1, E], F32)
                nc.vector.tensor_copy(gb_own, owncfs_ps)
                nc.vector.tensor_add(out=gb_own, in0=gb_own, in1=ownlin_ps)

                vals_b = route_sb.tile([1, 16], F32)
                idxb = route_sb.tile([1, 16], U32)
                tkb_scr = route_sb.tile([1, E], F32)
                topk16(gb_own, vals_b, idxb, tkb_scr)
                idxb_f = route_sb.tile([1, 16], F32)
                nc.vector.tensor_copy(idxb_f, idxb)
                brow = route_sb.tile([1, 16], F32)
                nc.vector.tensor_add(out=brow, in0=idxb_f, in1=iota64_f)
                boff_ps = route_ps.tile([16, 1], F32, tag="rps", name="boff_ps")
                nc.tensor.transpose(out=boff_ps, in_=brow, identity=ident[:1, :1])
                boff = route_sb.tile([16, 1], I32)
                nc.vector.tensor_copy(boff, boff_ps)

                nc.gpsimd.indirect_dma_start(
                    out=lora_b[:, :],
                    out_offset=None,
                    in_=btp_d[:, :],
                    in_offset=bass.IndirectOffsetOnAxis(ap=boff[:, 0:1], axis=0),
                )

            # ---------------- main pipeline ----------------
            # M1: aft[st] [16, 512] = sum_t lat16[:, t]^T @ xT[t][:, st]
            aft_ps = [
                main_ps_pool.tile([R, 512], F32, tag="mps", name=f"aft_ps{st}")
                for st in range(NST)
            ]
            # warm2 keeps PE continuously busy between DMA-paced tiles so the
            # pstate stays at full speed (idle PE falls back to the 2x-slower
            # mid pstate, which would make mm1 the M1 bottleneck)
            warm2 = main_ps_pool.tile([R, 512], F32, tag="mps", name="warm2")
            embedded = {
                1: lambda: load_cfs(0),
                2: lambda: load_cfs(1),
                6: cfs_and_collective,
                10: load_wrbt,
                14: gb_linear,
                24: router_b_tail,
            }
            for t in range(NIT):
                if t in embedded:
                    embedded[t]()
                xt_sb = x_pool.tile([P, S], F16, tag="x")
                nc.sync.dma_start(out=xt_sb, in_=xt_d[P * t : P * (t + 1), :])
                for st in range(NST):
                    nc.tensor.matmul(
                        out=aft_ps[st],
                        lhsT=lat16[:, 16 * t : 16 * (t + 1)],
                        rhs=xt_sb[:, 512 * st : 512 * (st + 1)],
                        start=(t == 0),
                        stop=(t == NIT - 1),
                    )
                if t >= 27:
                    nfill = 0  # drain the x-tile backlog at full mm pace
                elif t in embedded:
                    nfill = 2
                else:
                    nfill = 3
                _w = int(_os.environ.get("FILLW", "448"))
                for _ in range(nfill):
                    nc.tensor.matmul(
                        out=warm2[:, 0:_w],
                        lhsT=lat16[:, 0:16],
                        rhs=xt_sb[:, 0:_w],
                        start=True,
                        stop=True,
                    )
            aft16 = w_pool.tile([R, S], F16)

            def aft16_copy(st):
                if st % 2 == 0:
                    nc.vector.tensor_copy(
                        aft16[:, 512 * st : 512 * (st + 1)], aft_ps[st]
                    )
                else:
                    nc.scalar.activation(
                        aft16[:, 512 * st : 512 * (st + 1)],
                        aft_ps[st],
                        mybir.ActivationFunctionType.Copy,
                    )

            # M2: delta[s, o] = aft^T @ lora_b, written fp16 in half-rows;
            # each chunk's aft copy is emitted just before its subs so the
            # first sub isn't queued behind all four copies
            for st in range(NST):
                aft16_copy(st)
                for sub in range(4):
                    delta_sb = delta_pool.tile([P, OUT], F16, tag="d")
                    s0 = 512 * st + P * sub
                    for oc in range(NOC):
                        delta_ps = main_ps_pool.tile(
                            [P, 512], F32, tag="mps", name="delta_ps"
                        )
                        nc.tensor.matmul(
                            out=delta_ps,
                            lhsT=aft16[:, s0 : s0 + P],
                            rhs=lora_b[:, 512 * oc : 512 * (oc + 1)],
                            start=True,
                            stop=True,
                        )
                        # first sub alternates engines per oc so the first
                        # quarter-write's two copies run in parallel
                        _dve = (oc % 2 == 0) if (st == 0 and sub == 0) else (
                            oc % 4 < 2
                        )
                        if _dve:
                            nc.vector.tensor_copy(
                                delta_sb[:, 512 * oc : 512 * (oc + 1)], delta_ps
                            )
                        else:
                            nc.scalar.activation(
                                delta_sb[:, 512 * oc : 512 * (oc + 1)],
                                delta_ps,
                                mybir.ActivationFunctionType.Copy,
                            )
                        first = st == 0 and sub == 0
                        if first and oc % 2 == 1:
                            q = oc // 2
                            nc.sync.dma_start(
                                out=out_d[s0 : s0 + P, 1024 * q : 1024 * (q + 1)],
                                in_=delta_sb[:, 1024 * q : 1024 * (q + 1)],
                            )
                        elif not first and oc == 3:
                            nc.sync.dma_start(
                                out=out_d[s0 : s0 + P, 0 : OUT // 2],
                                in_=delta_sb[:, 0 : OUT // 2],
                            )
                    if not (st == 0 and sub == 0):
                        nc.sync.dma_start(
                            out=out_d[s0 : s0 + P, OUT // 2 :],
                            in_=delta_sb[:, OUT // 2 :],
                        )

    nc.compile()
    return nc


def build_core_maps(inputs):
    x = np.asarray(inputs["x"], dtype=np.float32)
    q = np.ascontiguousarray(inputs["query_signal"], dtype=np.float32)
    a_pool = np.ascontiguousarray(
        np.asarray(inputs["A_pool"], dtype=np.float32)
        .reshape(E * R, IN)
        .astype(np.float16)
    )
    btp = np.ascontiguousarray(
        np.asarray(inputs["B_pool"], dtype=np.float32)
        .transpose(2, 0, 1)
        .reshape(R * E, OUT)
        .astype(np.float16)
    )
    w_ra = np.asarray(inputs["W_rA"], dtype=np.float32)
    b_ra = np.ascontiguousarray(inputs["b_rA"], dtype=np.float32).reshape(E, 1)
    w_rb = np.asarray(inputs["W_rB"], dtype=np.float32)
    b_rb = np.ascontiguousarray(inputs["b_rB"], dtype=np.float32).reshape(E, 1)
    cfs = np.asarray(inputs["cfs_W"], dtype=np.float32)

    def sbuf_tiles(mat):
        # (IN, C) -> [P, NIT*C] with [p, t*C + c] = mat[t*128+p, c]
        c = mat.shape[1]
        return np.ascontiguousarray(
            mat.reshape(NIT, P, c).transpose(1, 0, 2).reshape(P, NIT * c)
        )

    qt = sbuf_tiles(q.T)
    qt16 = qt.astype(np.float16)
    wrat = sbuf_tiles(w_ra.T)
    wrbt = sbuf_tiles(w_rb.T).astype(np.float16)

    maps = []
    for c in range(B):
        onehot = np.zeros((B, 1), np.float32)
        onehot[c, 0] = 1.0
        sel64 = np.zeros((B * B, 1), np.float32)
        sel64[c::B, 0] = 1.0
        rsel16 = np.zeros((R, RPC), np.float32)
        rankrow = np.zeros((1, RPC), np.float32)
        for j in range(RPC):
            rsel16[RPC * c + j, j] = 16.0
            rankrow[0, j] = RPC * c + j
        cfs_c = np.ascontiguousarray(
            cfs[RPC * c : RPC * (c + 1)]
            .reshape(RPC, NIT, P, E)
            .transpose(0, 2, 1, 3)
            .reshape(RPC, P, NIT * E)
            .astype(np.float16)
        )
        xt = np.ascontiguousarray(x[c].T.astype(np.float16))
        maps.append(
            {
                "xt": xt,
                "qt": qt,
                "qt16": qt16,
                "wrat": wrat,
                "wrbt": wrbt,
                "cfs": cfs_c,
                "a_pool": a_pool,
                "btp": btp,
                "b_ra": b_ra,
                "b_rb": b_rb,
                "onehot": onehot,
                "sel64": sel64,
                "rsel16": rsel16,
                "rankrow": rankrow,
            }
        )
    return maps


def assemble_output(results):
    return np.stack([r["out"].astype(np.float32) for r in results], axis=0)


def kernel(_run_kwargs=None, **inputs: np.ndarray) -> np.ndarray:
    run_kwargs = _run_kwargs or {}
    nc = build_nc()
    in_maps = build_core_maps(inputs)
    res = run_bass_kernel_spmd(nc, in_maps, core_ids=list(range(B)), **run_kwargs)
    if run_kwargs:
        return res
    return assemble_output(res.results)
